# revision 2
# baseline (speedup 1.0000x reference)
"""GCN layer on 8 trn2 NeuronCores — v4.

out = segment_sum((h @ W * norm)[src], dst) * norm + bias

Algebra: (h@W)*norm = (h*norm)@W and segment_sum is linear, so
out = (segment_sum((h*norm)[src], dst) @ W) * norm + bias.
Gather RAW h rows (bf16) from HBM, apply norm[src] inside a bf16 one-hot
selection matrix, scatter-sum as PSUM-accumulated matmuls, then one
128x128 weight GEMM per output tile.

v4 vs baseline:
- h tables/msgs/one-hot/aggT/W in bf16 (halves gather DMA bytes, 4x PE
  matmul rate, 2x DVE). Final scale/bias/output stay f32.
- Gathers grouped over G slots but split into calls of <= KCAP indices
  (the SWDGE ring holds 1024 descriptors per queue and reclaim is broken
  on this runtime: total in-flight descriptors must stay under capacity).
- Optional multi-queue SWDGE (parallel Q7 descriptor generation).
- PSUM->SBUF copies + norm scale on the Activation engine, bias add on
  DVE, freeing DVE for one-hot builds.

Sharding: nodes padded to 784 tiles of 128; edges partitioned by dst
tile; tiles rank-dealt across 8 cores (one shared SPMD program). h is
split into 4 quadrant tables of 25088 rows for int16 gather indices.
"""
import numpy as np

import concourse.bass as bass
import concourse.mybir as mybir
import concourse.tile as tile
from concourse import bacc
from concourse.bass_utils import run_bass_kernel_spmd
from concourse.library_config import mlp

P = 128
N = 100000
E = 1600000
NCORES = 8
NT = 784
NPAD = NT * P
TPC = NT // NCORES      # 98 slots per core
Q = 4
R = NPAD // Q           # 25088
G = 7                   # slots per group
NG = TPC // G

KCAP = 1024             # max idxs per dma_gather call (ring safety)
NQUEUES = 4             # SWDGE queues to spread gather calls over
SCRATCH = 16384         # dynamic DMA scratch bytes/partition

_cache = {}
RUN_KWARGS = {}
LAST_RESULTS = None
EMULATE = False
LAST_NC = None
LAST_IN_MAPS = None


def _group_layout(K_sq, Csq):
    """Chunk layout per group: quadrant-major, slot-minor; plus gather
    call splits of <= KCAP indices per call."""
    Cg, runs, Kgq, calls = [], [], [], []
    for g in range(NG):
        slots = range(g * G, (g + 1) * G)
        off = 0
        r = [[] for _ in range(G)]
        kq = []
        cl = []
        for q in range(Q):
            Kq = sum(K_sq[s][q] for s in slots)
            kq.append(Kq)
            # split this quadrant's gather into calls of <= KCAP
            qcalls = []
            left = Kq
            while left > 0:
                k = min(KCAP, left)
                qcalls.append(k)
                left -= k
            cl.append(qcalls)
            for j, s in enumerate(slots):
                cqs = Csq[s][q]
                if cqs:
                    r[j].append((off, cqs))
                    off += cqs
        Cg.append(off)
        runs.append(r)
        Kgq.append(kq)
        calls.append(cl)
    return Cg, runs, Kgq, calls


def _build_program(K_sq, Csq):
    Cg, runs, Kgq, calls = _group_layout(K_sq, Csq)
    Cmax = max(Cg)
    chunk_cols = sum(Cg)
    idx_cols = sum(sum(kq) for kq in Kgq) // 16

    nc = bacc.Bacc(None, target_bir_lowering=False,
                   dynamic_dma_scratch_size=SCRATCH,
                   num_swdge_queues=NQUEUES)
    f32 = mybir.dt.float32
    bf16 = mybir.dt.bfloat16
    hq_d = [nc.dram_tensor(f"h{q}", [R, P], bf16, kind="ExternalInput")
            for q in range(Q)]
    idx_d = nc.dram_tensor("idx16", [P, idx_cols], mybir.dt.int16,
                           kind="ExternalInput")
    dstl_d = nc.dram_tensor("dstl", [P, chunk_cols], f32, kind="ExternalInput")
    ew_d = nc.dram_tensor("ew", [P, chunk_cols], f32, kind="ExternalInput")
    ncol_d = nc.dram_tensor("ncol", [P, TPC], f32, kind="ExternalInput")
    bb_d = nc.dram_tensor("bb", [P, P], f32, kind="ExternalInput")
    w_d = nc.dram_tensor("wt", [P, P], bf16, kind="ExternalInput")
    out_d = nc.dram_tensor("out", [TPC * P, P], f32, kind="ExternalOutput")
    out_v = out_d.rearrange("(t p) d -> t p d", p=P)

    with tile.TileContext(nc) as tc:
        with (
            tc.tile_pool(name="const", bufs=1) as cpool,
            tc.tile_pool(name="gather", bufs=2) as gpool,
            tc.tile_pool(name="pt", bufs=4) as ptpool,
            tc.tile_pool(name="ps", bufs=2, space="PSUM") as pspool,
            tc.tile_pool(name="ps2", bufs=2, space="PSUM") as ps2pool,
            tc.tile_pool(name="oo", bufs=3) as opool,
            tc.tile_pool(name="agg", bufs=2) as aggpool,
        ):
            nc.gpsimd.load_library(mlp)
            idx_sb = cpool.tile([P, idx_cols], mybir.dt.int16)
            nc.sync.dma_start(idx_sb[:], idx_d[:])
            dstl_sb = cpool.tile([P, chunk_cols], f32)
            nc.sync.dma_start(dstl_sb[:], dstl_d[:])
            ew_sb = cpool.tile([P, chunk_cols], f32)
            nc.sync.dma_start(ew_sb[:], ew_d[:])
            ncol_sb = cpool.tile([P, TPC], f32)
            nc.sync.dma_start(ncol_sb[:], ncol_d[:])
            bb_sb = cpool.tile([P, P], f32)
            nc.sync.dma_start(bb_sb[:], bb_d[:])
            w_sb = cpool.tile([P, P], bf16)
            nc.sync.dma_start(w_sb[:], w_d[:])
            iota_i = cpool.tile([P, P], mybir.dt.int32)
            nc.gpsimd.iota(iota_i[:], pattern=[[1, P]], base=0,
                           channel_multiplier=0)
            iota_b = cpool.tile([P, P], bf16)
            nc.vector.tensor_copy(iota_b[:], iota_i[:])

            ioff = 0
            goff = 0
            callno = 0
            for g in range(NG):
                msgs = gpool.tile([P, Cmax, P], bf16, tag="msgs")
                local = 0
                for q in range(Q):
                    for K in calls[g][q]:
                        cq = K // P
                        nc.gpsimd.dma_gather(
                            msgs[:, local:local + cq, :], hq_d[q][:],
                            idx_sb[:, ioff:ioff + K // 16], K, K, P,
                            queue_num=callno % NQUEUES,
                        )
                        callno += 1
                        local += cq
                        ioff += K // 16
                for j in range(G):
                    s = g * G + j
                    nchunks = sum(n for _, n in runs[g][j])
                    aggT_ps = pspool.tile([P, P], f32, tag="agg")
                    ci = 0
                    for c0, n in runs[g][j]:
                        for c in range(c0, c0 + n):
                            col = goff + c
                            pt = ptpool.tile([P, P], bf16, tag="pt")
                            nc.vector.tensor_scalar(
                                pt[:], iota_b[:],
                                dstl_sb[:, col:col + 1],
                                ew_sb[:, col:col + 1],
                                op0=mybir.AluOpType.is_equal,
                                op1=mybir.AluOpType.mult,
                            )
                            nc.tensor.matmul(
                                aggT_ps[:], lhsT=msgs[:, c, :], rhs=pt[:],
                                start=(ci == 0), stop=(ci == nchunks - 1),
                            )
                            ci += 1
                    aggT_sb = aggpool.tile([P, P], bf16, tag="aggT")
                    nc.scalar.activation(aggT_sb[:], aggT_ps[:],
                                         mybir.ActivationFunctionType.Copy)
                    out_ps = ps2pool.tile([P, P], f32, tag="out")
                    nc.tensor.matmul(out_ps[:], lhsT=aggT_sb[:], rhs=w_sb[:],
                                     start=True, stop=True)
                    o_sb = opool.tile([P, P], f32, tag="o")
                    nc.scalar.activation(o_sb[:], out_ps[:],
                                         mybir.ActivationFunctionType.Copy,
                                         scale=ncol_sb[:, s:s + 1])
                    nc.vector.tensor_tensor(o_sb[:], o_sb[:], bb_sb[:],
                                            op=mybir.AluOpType.add)
                    nc.sync.dma_start(out_v[s], o_sb[:])
                goff += Cg[g]
    nc.compile()
    return nc


def _host_prep(h, norm, src, dst, weight, bias):
    import ml_dtypes
    bf16 = ml_dtypes.bfloat16

    h_pad = np.zeros((NPAD, P), np.float32)
    h_pad[:N] = h
    hq = [np.ascontiguousarray(h_pad[q * R:(q + 1) * R]).astype(bf16)
          for q in range(Q)]
    norm_pad = np.zeros((NPAD,), np.float32)
    norm_pad[:N] = norm

    tile_id = dst // P
    dstl_all = (dst % P).astype(np.float32)
    quad = src // R
    srcl_all = (src % R).astype(np.int16)
    ew_all = norm[src].astype(np.float32)

    key = tile_id * Q + quad
    order = np.argsort(key, kind="stable")
    counts = np.bincount(key, minlength=NT * Q).reshape(NT, Q)
    starts = np.zeros((NT, Q), np.int64)
    starts.reshape(-1)[1:] = np.cumsum(counts.reshape(-1))[:-1]

    totals = counts.sum(1)
    rank = np.argsort(-totals, kind="stable")
    tiles_sc = rank.reshape(TPC, NCORES)

    cnt_sc = counts[tiles_sc]
    K_sq = ((cnt_sc.max(axis=1) + P - 1) // P * P).astype(np.int64)
    Csq = (K_sq // P).astype(np.int64)

    Cg, runs, Kgq, calls = _group_layout(K_sq.tolist(), Csq.tolist())
    chunk_cols = int(sum(Cg))
    idx_cols = int(sum(sum(kq) for kq in Kgq) // 16)

    srcl_ord = srcl_all[order]
    dstl_ord = dstl_all[order]
    ew_ord = ew_all[order]

    in_maps = []
    for c in range(NCORES):
        idx16 = np.zeros((P, idx_cols), np.int16)
        dstl_a = np.zeros((P, chunk_cols), np.float32)
        ew_a = np.zeros((P, chunk_cols), np.float32)
        ioff = 0
        goff = 0
        for g in range(NG):
            coff = goff
            for q in range(Q):
                Kg = int(Kgq[g][q])
                if Kg == 0:
                    continue
                seg_src = np.zeros((Kg,), np.int16)
                pos = 0
                for s in range(g * G, (g + 1) * G):
                    K = int(K_sq[s, q])
                    if K == 0:
                        continue
                    t = tiles_sc[s, c]
                    cnt = int(counts[t, q])
                    st = int(starts[t, q])
                    seg_src[pos:pos + cnt] = srcl_ord[st:st + cnt]
                    cq = int(Csq[s, q])
                    seg_dstl = np.zeros((cq * P,), np.float32)
                    seg_dstl[:cnt] = dstl_ord[st:st + cnt]
                    seg_ew = np.zeros((cq * P,), np.float32)
                    seg_ew[:cnt] = ew_ord[st:st + cnt]
                    dstl_a[:, coff:coff + cq] = seg_dstl.reshape(cq, P).T
                    ew_a[:, coff:coff + cq] = seg_ew.reshape(cq, P).T
                    coff += cq
                    pos += K
                # wrap indices PER GATHER CALL
                cpos = 0
                for K in calls[g][q]:
                    segc = seg_src[cpos:cpos + K]
                    wrapped = segc.reshape(K // 16, 16).T
                    idx16[:, ioff:ioff + K // 16] = np.tile(wrapped, (8, 1))
                    ioff += K // 16
                    cpos += K
            goff += int(Cg[g])
        node_ids = tiles_sc[:, c][:, None] * P + np.arange(P)[None, :]
        ncol = norm_pad[node_ids].T.astype(np.float32).copy()
        in_maps.append({
            "h0": hq[0], "h1": hq[1], "h2": hq[2], "h3": hq[3],
            "idx16": idx16,
            "dstl": dstl_a, "ew": ew_a,
            "ncol": np.ascontiguousarray(ncol),
            "bb": np.tile(bias[None, :], (P, 1)).astype(np.float32),
            "wt": weight.astype(bf16),
        })
    return K_sq, Csq, in_maps, tiles_sc


def kernel(h, norm, src, dst, weight, bias):
    h = np.ascontiguousarray(h, dtype=np.float32)
    norm = np.ascontiguousarray(norm, dtype=np.float32).reshape(-1)
    src = np.ascontiguousarray(src, dtype=np.int64).reshape(-1)
    dst = np.ascontiguousarray(dst, dtype=np.int64).reshape(-1)
    weight = np.ascontiguousarray(weight, dtype=np.float32)
    bias = np.ascontiguousarray(bias, dtype=np.float32).reshape(-1)
    n, d = h.shape
    e = src.shape[0]
    assert (n, d, e) == (N, P, E), (n, d, e)

    K_sq, Csq, in_maps, tiles_sc = _host_prep(h, norm, src, dst, weight, bias)

    if not EMULATE:
        key_prog = (tuple(map(tuple, K_sq)),)
        if key_prog not in _cache:
            _cache[key_prog] = _build_program(K_sq.tolist(), Csq.tolist())
        nc = _cache[key_prog]

    global LAST_NC, LAST_IN_MAPS
    LAST_NC, LAST_IN_MAPS = (nc if not EMULATE else None), in_maps
    if EMULATE:
        results = [_emulate_core(m, K_sq, Csq) for m in in_maps]
    else:
        res = run_bass_kernel_spmd(nc, in_maps, core_ids=list(range(NCORES)),
                                   **RUN_KWARGS)
        global LAST_RESULTS
        LAST_RESULTS = res
        results = [res.results[c]["out"] for c in range(NCORES)]

    out_tiles = np.zeros((NT, P, P), np.float32)
    for c in range(NCORES):
        out_tiles[tiles_sc[:, c]] = results[c].reshape(TPC, P, P)
    return out_tiles.reshape(NPAD, P)[:N].copy()


def _emulate_core(m, K_sq, Csq):
    import ml_dtypes
    bf16 = ml_dtypes.bfloat16
    Cg, runs, Kgq, calls = _group_layout(K_sq.tolist(), Csq.tolist())
    hq = [m[f"h{q}"] for q in range(Q)]
    iota = np.arange(P, dtype=np.float32)[None, :]
    out = np.zeros((TPC, P, P), np.float32)
    ioff = 0
    goff = 0
    for g in range(NG):
        chunks = []
        for q in range(Q):
            for K in calls[g][q]:
                idx = m["idx16"][:16, ioff:ioff + K // 16].T.reshape(-1)
                gat = hq[q][idx]
                chunks.append(gat.reshape(K // P, P, P))
                ioff += K // 16
        msgs = np.concatenate(chunks, axis=0)
        for j in range(G):
            s = g * G + j
            aggT = np.zeros((P, P), np.float32)
            for c0, n in runs[g][j]:
                for c in range(c0, c0 + n):
                    col = goff + c
                    dstl = m["dstl"][:, col].astype(np.float32)[:, None]
                    ew = m["ew"][:, col].astype(np.float32)[:, None]
                    pt = ((iota == dstl).astype(np.float32) * ew).astype(bf16)
                    aggT += msgs[c].astype(np.float32).T @ \
                        pt.astype(np.float32)
            aggT = aggT.astype(bf16).astype(np.float32)
            o = aggT.T @ m["wt"].astype(np.float32)
            o = o * m["ncol"][:, s][:, None] + m["bb"]
            out[s] = o
        goff += int(Cg[g])
    return out
